# revision 11
# baseline (speedup 1.0000x reference)
"""Trainium2 Bass kernel: 49-tap separable Gaussian blur (sigma=3) on
[64, 512, 512, 3] f32 NHWC, data-parallel over 8 NeuronCores (8 images each).

Algorithm per image (on-chip):
  view image as X[h, (w,c)] = [512, 1536].
  Pass 1 (blur along H), "data-stationary" matmul form that transposes for free:
      Y1[(c,w), h] = sum_h' X[h', (c,w)] * A[h', h]
    where A is the 512x512 banded symmetric Toeplitz blur matrix
    (A[i,j] = g[j-i+24], zero outside the 49-band -> jax 'SAME' zero padding).
    lhsT = X tile [128 h', 128 (c,w)] (stationary), rhs = A row-block [128, band].
    The (w,c)->(c,w) reorder happens for free in the f32->bf16 cast copy, so the
    (c,w) partition tiles of Y1 are single-channel w-tiles and pass 2 can reuse A.
  Pass 2 (blur along W), same trick on Y1:
      Z[h, (c,w)] = sum_w' Y1[(c,w'), h] * A[w', w]
    Output PSUM tiles are [128 h, 512 w] per channel; the PSUM->SBUF eviction
    copy scatters (c,w)->(w,c) (stride-12B writes, free on ACT/DVE) so the
    output DMA is fully contiguous NHWC.

Contraction is banded: each 128-row block of A only touches 176 output columns,
so each PSUM tile takes 4 matmuls (one full-width N=512 with start=True to
zero-init via A's zero entries, then 3 narrow accumulating bands).

Everything is bf16 on the TensorEngine (f32 PSUM accumulate); HBM traffic is
the f32 in/out (2 x 3 MB per image) => memory-bound at ~358 GB/s per core.
"""

import os

import numpy as np
import ml_dtypes

import concourse.bass as bass
import concourse.mybir as mybir
import concourse.tile as tile
from concourse import bacc
from concourse.bass_utils import run_bass_kernel_spmd

# build-config knobs for debugging (env-settable from test harnesses)
IO_DMA_ENGINE = os.environ.get("BLUR_IO_DMA", "sync")  # sync | gpsimd
# (gpsimd SWDGE for the I/O-tensor DMAs crashes the exec unit on HW —
#  NRT_EXEC_UNIT_UNRECOVERABLE; sync HWDGE works once Bacc legalizes waits)
CAST_ENGINE = os.environ.get("BLUR_CAST", "gpsimd")      # gpsimd | vector
EVICT2_ENGINE = os.environ.get("BLUR_EVICT2", "scalar")  # scalar | vector
BANDED = os.environ.get("BLUR_BANDED", "1") == "1"

KSIZE = 49
SIGMA = 3.0
R = (KSIZE - 1) // 2  # 24
H = 512
W = 512
C = 3
WC = W * C  # 1536
P = 128
HT = H // P  # 4 partition tiles per 512 dim
N_CORES = 8
IMGS = 8  # images per core

_CACHE: dict = {}


def _gauss_matrix() -> np.ndarray:
    """512x512 banded symmetric blur matrix A[i, j] = g[j - i + 24]."""
    r = np.arange(KSIZE, dtype=np.float32) - (KSIZE - 1) / 2.0
    g = np.exp(-(r * r) / (2.0 * SIGMA * SIGMA)).astype(np.float32)
    g = g / g.sum(dtype=np.float32)
    A = np.zeros((H, H), dtype=np.float32)
    for i in range(H):
        lo, hi = max(0, i - R), min(H, i + R + 1)
        A[i, lo:hi] = g[lo - i + R : hi - i + R]
    return A


def _bands():
    """Output-column band written by each 128-row block of A.

    Block t=0 streams the full 512 columns with start=True: its rows are zero
    outside [0, 152), so the extra columns write exact zeros, which zero-
    initializes the whole PSUM bank for the later accumulating bands.
    """
    bands = []
    for t in range(HT):
        if t == 0 or not BANDED:
            bands.append((0, H))
        else:
            bands.append((P * t - R, min(H, P * t + P + R)))
    return bands


def _build():
    # Bacc (not raw Bass): its compile() legalizes multi-sem waits down to
    # the 1-wait-per-instruction HW limit (generate_event_semaphores).
    nc = bacc.Bacc("TRN2", target_bir_lowering=False, debug=False,
                   num_devices=N_CORES)
    x_ext = nc.declare_dram_parameter("x", [IMGS, H, WC], mybir.dt.float32,
                                      isOutput=False)
    out_ext = nc.declare_dram_parameter("out", [IMGS, H, WC], mybir.dt.float32,
                                        isOutput=True)
    a_np = _gauss_matrix().astype(ml_dtypes.bfloat16)
    a_dram = nc.inline_tensor(a_np, name="gmat")

    bands = _bands()
    x_ap = x_ext[:].rearrange("n (t p) f -> n p t f", p=P)
    out_ap = out_ext[:].rearrange("n (t p) f -> n p t f", p=P)

    with tile.TileContext(nc) as tc:
        from contextlib import ExitStack

        with ExitStack() as ctx:
            const_pool = ctx.enter_context(tc.tile_pool(name="const", bufs=1))
            x32_pool = ctx.enter_context(tc.tile_pool(name="x32p", bufs=2))
            x16_pool = ctx.enter_context(tc.tile_pool(name="x16p", bufs=2))
            y1_pool = ctx.enter_context(tc.tile_pool(name="y1p", bufs=2))
            z_pool = ctx.enter_context(tc.tile_pool(name="zp", bufs=2))
            ps1_pool = ctx.enter_context(
                tc.tile_pool(name="ps1p", bufs=3, space="PSUM"))
            ps2_pool = ctx.enter_context(
                tc.tile_pool(name="ps2p", bufs=3, space="PSUM"))

            # A row-blocks: g_sb[:, t, :] = A[128t : 128t+128, :]
            g_sb = const_pool.tile([P, HT, H], mybir.dt.bfloat16)
            nc.sync.dma_start(
                out=g_sb[:], in_=a_dram[:].rearrange("(t p) h -> p t h", p=P))

            for n in range(IMGS):
                # ---- load image, cast f32->bf16 with (w,c)->(c,w) reorder
                x32 = x32_pool.tile([P, HT, WC], mybir.dt.float32)
                io_dma = getattr(nc, IO_DMA_ENGINE)
                io_dma.dma_start(out=x32[:], in_=x_ap[n])
                x16 = x16_pool.tile([P, HT, C, W], mybir.dt.bfloat16)
                cast_eng = getattr(nc, CAST_ENGINE)
                cast_eng.tensor_copy(
                    x16[:].rearrange("p t c w -> p t w c"),
                    x32[:].rearrange("p t (w c) -> p t w c", c=C),
                )

                # ---- pass 1: blur along H; output Y1[(c,w), h] transposed
                y1 = y1_pool.tile([P, C, HT, H], mybir.dt.bfloat16)
                for c in range(C):
                    for wt in range(HT):
                        ps1 = ps1_pool.tile([P, H], mybir.dt.float32)
                        for t in range(HT):
                            b0, b1 = bands[t]
                            nc.tensor.matmul(
                                ps1[:, b0:b1],
                                lhsT=x16[:, t, c, wt * P:(wt + 1) * P],
                                rhs=g_sb[:, t, b0:b1],
                                start=(t == 0),
                                stop=(t == HT - 1),
                            )
                        nc.vector.tensor_copy(y1[:, c, wt, :], ps1[:])

                # ---- pass 2: blur along W; output Z[h, (w,c)] NHWC-ready
                z = z_pool.tile([P, HT, WC], mybir.dt.float32)
                for ht in range(HT):
                    for c in range(C):
                        ps2 = ps2_pool.tile([P, W], mybir.dt.float32)
                        for t in range(HT):
                            b0, b1 = bands[t]
                            nc.tensor.matmul(
                                ps2[:, b0:b1],
                                lhsT=y1[:, c, t, ht * P:(ht + 1) * P],
                                rhs=g_sb[:, t, b0:b1],
                                start=(t == 0),
                                stop=(t == HT - 1),
                            )
                        # eviction scatters channel c into interleaved (w,c)
                        zdst = z[:, ht].rearrange("p (w c) -> p c w", c=C)[:, c, :]
                        if EVICT2_ENGINE == "scalar":
                            nc.scalar.activation(
                                zdst, ps2[:], mybir.ActivationFunctionType.Copy)
                        else:
                            nc.vector.tensor_copy(zdst, ps2[:])
                io_dma.dma_start(out=out_ap[n], in_=z[:])

    nc.compile()
    return nc


def kernel(x: np.ndarray) -> np.ndarray:
    assert x.shape == (N_CORES * IMGS, H, W, C) and x.dtype == np.float32
    if "nc" not in _CACHE:
        _CACHE["nc"] = _build()
    nc = _CACHE["nc"]

    x = np.ascontiguousarray(x)
    in_maps = [
        {"x": x[i * IMGS:(i + 1) * IMGS].reshape(IMGS, H, WC)}
        for i in range(N_CORES)
    ]
    trace = os.environ.get("BLUR_TRACE", "0") == "1"
    res = run_bass_kernel_spmd(nc, in_maps, core_ids=list(range(N_CORES)),
                               trace=trace)
    _CACHE["last_results"] = res
    out = np.concatenate([res.results[i]["out"] for i in range(N_CORES)], axis=0)
    return np.ascontiguousarray(out.reshape(N_CORES * IMGS, H, W, C))


if __name__ == "__main__":
    xs = np.random.randn(64, H, W, C).astype(np.float32)
    y = kernel(xs)
    print(y.shape, y.dtype)


# revision 19
# speedup vs baseline: 2.0490x; 2.0490x over previous
"""Trainium2 Bass kernel: 49-tap separable Gaussian blur (sigma=3) on
[64, 512, 512, 3] f32 NHWC, data-parallel over 8 NeuronCores (8 images each).

Algorithm per image (on-chip), all matmuls in float32r (1 cycle/row at N>=256):
  view image as X[h, (w,c)] = [512, 1536].
  Pass 1 (blur along H), "data-stationary" matmul form that transposes for free:
      Y1[(c,w), h] = sum_h' X[h', (c,w)] * A[h', h]
    where A is the 512x512 banded symmetric Toeplitz blur matrix
    (A[i,j] = g[j-i+24], zero outside the 49-band == jax 'SAME' zero padding).
    lhsT = X tile [128 h', 128 (c,w)-strided] (stationary), rhs = A row-block.
    The lhsT free-dim AP walks w with stride 12B at offset 4c, so Y1's
    partition tiles are single-channel w-tiles and pass 2 reuses the same A.
  Pass 2 (blur along W), same trick on Y1:
      Z[h, (c,w)] = sum_w' Y1[(c,w'), h] * A[w', w]
    Output PSUM tiles are [128 h, 512 w] per channel; the PSUM->SBUF eviction
    copy scatters (c,w)->(w,c) (stride-12B writes, free on ACT/DVE) so the
    output DMA is fully contiguous NHWC.

Contraction is banded: each 128-row block of A only touches 176 output
columns. Block t=0 streams the full 512 columns with start=True (its zero
entries zero-initialize the whole PSUM bank); blocks t>=1 stream a 256-wide
window covering their band (width >=256 keeps float32r at full rate; the
extra columns multiply zero entries of A and accumulate 0).

HBM traffic is the f32 in/out (2 x 3 MB per image) => memory-bound at
~358 GB/s per core.
"""

import os

import numpy as np

import concourse.mybir as mybir
import concourse.tile as tile
from concourse import bacc
from concourse.bass_utils import run_bass_kernel_spmd

# build-config knobs for debugging (env-settable from test harnesses)
IO_DMA_ENGINE = os.environ.get("BLUR_IO_DMA", "sync")  # sync | gpsimd
# (gpsimd SWDGE for the I/O-tensor DMAs crashes the exec unit on HW —
#  NRT_EXEC_UNIT_UNRECOVERABLE; sync HWDGE works once Bacc legalizes waits)
EVICT2_ENGINE = os.environ.get("BLUR_EVICT2", "scalar")  # scalar | vector
# bfloat16: contiguous DVE cast (2x mode) + stride-6B weight APs.
# float32r needs explicitly-rounded inputs (extra pass) + 256-wide bands
# (+26% PE stream) — net loss; float32 streams at 1/4 rate.
MM_DTYPE = os.environ.get("BLUR_MM_DTYPE", "bfloat16")

KSIZE = 49
SIGMA = 3.0
R = (KSIZE - 1) // 2  # 24
H = 512
W = 512
C = 3
WC = W * C  # 1536
P = 128
HT = H // P  # 4 partition tiles per 512 dim
N_CORES = 8
IMGS = 8  # images per core
# float32r streams at 1/4 rate below 256-wide output; bf16 has no minimum
MIN_N = 256 if "float32r" in MM_DTYPE else 1

_CACHE: dict = {}


def _gauss_matrix() -> np.ndarray:
    """512x512 banded symmetric blur matrix A[i, j] = g[j - i + 24]."""
    r = np.arange(KSIZE, dtype=np.float32) - (KSIZE - 1) / 2.0
    g = np.exp(-(r * r) / (2.0 * SIGMA * SIGMA)).astype(np.float32)
    g = g / g.sum(dtype=np.float32)
    A = np.zeros((H, H), dtype=np.float32)
    for i in range(H):
        lo, hi = max(0, i - R), min(H, i + R + 1)
        A[i, lo:hi] = g[lo - i + R : hi - i + R]
    return A


def _bands():
    """Output-column window streamed for each 128-row block of A."""
    bands = [(0, H)]  # t=0: full width, start=True zero-fills the bank
    for t in range(1, HT):
        b0 = P * t - R
        b1 = min(H, P * t + P + R)  # true band, width <= 176
        if b1 - b0 < MIN_N:  # widen leftward (extra cols hit zeros of A)
            b0 = max(0, b1 - MIN_N)
        bands.append((b0, b1))
    return bands


def _build():
    # Bacc (not raw Bass): its compile() legalizes multi-sem waits down to
    # the 1-wait-per-instruction HW limit (generate_event_semaphores).
    nc = bacc.Bacc("TRN2", target_bir_lowering=False, debug=False,
                   num_devices=N_CORES)
    x_ext = nc.declare_dram_parameter("x", [IMGS, H, WC], mybir.dt.float32,
                                      isOutput=False)
    out_ext = nc.declare_dram_parameter("out", [IMGS, H, WC], mybir.dt.float32,
                                        isOutput=True)
    mm_dt = getattr(mybir.dt, MM_DTYPE)
    a_np = _gauss_matrix()
    if MM_DTYPE == "bfloat16":
        import ml_dtypes
        a_np = a_np.astype(ml_dtypes.bfloat16)
    a_dram = nc.inline_tensor(a_np, name="gmat")
    bands = _bands()
    x_ap = x_ext[:].rearrange("n (t p) f -> n p t f", p=P)
    out_ap = out_ext[:].rearrange("n (t p) f -> n p t f", p=P)

    with tile.TileContext(nc) as tc:
        from contextlib import ExitStack

        with ExitStack() as ctx:
            const_pool = ctx.enter_context(tc.tile_pool(name="const", bufs=1))
            x32_pool = ctx.enter_context(tc.tile_pool(name="x32p", bufs=3))
            x16_pool = ctx.enter_context(tc.tile_pool(name="x16p", bufs=2))
            y1_pool = ctx.enter_context(tc.tile_pool(name="y1p", bufs=2))
            z_pool = ctx.enter_context(tc.tile_pool(name="zp", bufs=2))
            ps1_pool = ctx.enter_context(
                tc.tile_pool(name="ps1p", bufs=4, space="PSUM"))
            ps2_pool = ctx.enter_context(
                tc.tile_pool(name="ps2p", bufs=4, space="PSUM"))

            # A row-blocks: g_sb[:, t, :] = A[128t : 128t+128, :]
            g_sb = const_pool.tile([P, HT, H], mm_dt)
            nc.sync.dma_start(
                out=g_sb[:], in_=a_dram[:].rearrange("(t p) h -> p t h", p=P))

            io_dma = getattr(nc, IO_DMA_ENGINE)
            for n in range(IMGS):
                x32 = x32_pool.tile([P, HT, WC], mybir.dt.float32)
                io_dma.dma_start(out=x32[:], in_=x_ap[n])
                # contiguous f32->bf16 cast (DVE 2x mode); layout stays (w,c)
                x16 = x16_pool.tile([P, HT, WC], mm_dt)
                nc.vector.tensor_copy(x16[:], x32[:])
                # natural-layout view for strided (c,w) weight slices:
                # [p, t, w, c] -> lhsT free dim walks w (stride 6B) at fixed c
                x16v = x16[:].rearrange("p t (w c) -> p t w c", c=C)

                # ---- pass 1: blur along H; output Y1[(c,w), h] transposed
                y1 = y1_pool.tile([P, C, HT, H], mm_dt)
                for c in range(C):
                    for wt in range(HT):
                        ps1 = ps1_pool.tile([P, H], mybir.dt.float32)
                        for t in range(HT):
                            b0, b1 = bands[t]
                            nc.tensor.matmul(
                                ps1[:, b0:b1],
                                lhsT=x16v[:, t, wt * P:(wt + 1) * P, c],
                                rhs=g_sb[:, t, b0:b1],
                                start=(t == 0),
                                stop=(t == HT - 1),
                            )
                        nc.vector.tensor_copy(y1[:, c, wt, :], ps1[:])

                # ---- pass 2: blur along W; output Z[h, (w,c)] NHWC-ready
                z = z_pool.tile([P, HT, WC], mybir.dt.float32)
                for ht in range(HT):
                    for c in range(C):
                        ps2 = ps2_pool.tile([P, W], mybir.dt.float32)
                        for t in range(HT):
                            b0, b1 = bands[t]
                            nc.tensor.matmul(
                                ps2[:, b0:b1],
                                lhsT=y1[:, c, t, ht * P:(ht + 1) * P],
                                rhs=g_sb[:, t, b0:b1],
                                start=(t == 0),
                                stop=(t == HT - 1),
                            )
                        # eviction scatters channel c into interleaved (w,c)
                        zdst = z[:, ht].rearrange("p (w c) -> p c w", c=C)[:, c, :]
                        if EVICT2_ENGINE == "scalar":
                            nc.scalar.activation(
                                zdst, ps2[:], mybir.ActivationFunctionType.Copy)
                        else:
                            nc.vector.tensor_copy(zdst, ps2[:])
                io_dma.dma_start(out=out_ap[n], in_=z[:])

    nc.compile()
    return nc


def kernel(x: np.ndarray) -> np.ndarray:
    assert x.shape == (N_CORES * IMGS, H, W, C) and x.dtype == np.float32
    if "nc" not in _CACHE:
        _CACHE["nc"] = _build()
    nc = _CACHE["nc"]

    x = np.ascontiguousarray(x)
    in_maps = [
        {"x": x[i * IMGS:(i + 1) * IMGS].reshape(IMGS, H, WC)}
        for i in range(N_CORES)
    ]
    trace = os.environ.get("BLUR_TRACE", "0") == "1"
    res = run_bass_kernel_spmd(nc, in_maps, core_ids=list(range(N_CORES)),
                               trace=trace)
    _CACHE["last_results"] = res
    out = np.concatenate([res.results[i]["out"] for i in range(N_CORES)], axis=0)
    return np.ascontiguousarray(out.reshape(N_CORES * IMGS, H, W, C))


if __name__ == "__main__":
    xs = np.random.randn(64, H, W, C).astype(np.float32)
    y = kernel(xs)
    print(y.shape, y.dtype)


# revision 24
# speedup vs baseline: 2.5014x; 1.2208x over previous
"""Trainium2 Bass kernel: 49-tap separable Gaussian blur (sigma=3) on
[64, 512, 512, 3] f32 NHWC, data-parallel over 8 NeuronCores (8 images each).

Algorithm per image (on-chip), all matmuls in float32r (1 cycle/row at N>=256):
  view image as X[h, (w,c)] = [512, 1536].
  Pass 1 (blur along H), "data-stationary" matmul form that transposes for free:
      Y1[(c,w), h] = sum_h' X[h', (c,w)] * A[h', h]
    where A is the 512x512 banded symmetric Toeplitz blur matrix
    (A[i,j] = g[j-i+24], zero outside the 49-band == jax 'SAME' zero padding).
    lhsT = X tile [128 h', 128 (c,w)-strided] (stationary), rhs = A row-block.
    The lhsT free-dim AP walks w with stride 12B at offset 4c, so Y1's
    partition tiles are single-channel w-tiles and pass 2 reuses the same A.
  Pass 2 (blur along W), same trick on Y1:
      Z[h, (c,w)] = sum_w' Y1[(c,w'), h] * A[w', w]
    Output PSUM tiles are [128 h, 512 w] per channel; the PSUM->SBUF eviction
    copy scatters (c,w)->(w,c) (stride-12B writes, free on ACT/DVE) so the
    output DMA is fully contiguous NHWC.

Contraction is banded: each 128-row block of A only touches 176 output
columns. Block t=0 streams the full 512 columns with start=True (its zero
entries zero-initialize the whole PSUM bank); blocks t>=1 stream a 256-wide
window covering their band (width >=256 keeps float32r at full rate; the
extra columns multiply zero entries of A and accumulate 0).

HBM traffic is the f32 in/out (2 x 3 MB per image) => memory-bound at
~358 GB/s per core.
"""

import os

import numpy as np

import concourse.mybir as mybir
import concourse.tile as tile
from concourse import bacc
from concourse.bass_utils import run_bass_kernel_spmd

# build-config knobs for debugging (env-settable from test harnesses)
IO_DMA_ENGINE = os.environ.get("BLUR_IO_DMA", "sync")  # sync | gpsimd
# (gpsimd SWDGE for the I/O-tensor DMAs crashes the exec unit on HW —
#  NRT_EXEC_UNIT_UNRECOVERABLE; sync HWDGE works once Bacc legalizes waits)
EVICT2_ENGINE = os.environ.get("BLUR_EVICT2", "scalar")  # scalar | vector
# bfloat16: contiguous DVE cast (2x mode) + stride-6B weight APs.
# float32r needs explicitly-rounded inputs (extra pass) + 256-wide bands
# (+26% PE stream) — net loss; float32 streams at 1/4 rate.
MM_DTYPE = os.environ.get("BLUR_MM_DTYPE", "bfloat16")

KSIZE = 49
SIGMA = 3.0
R = (KSIZE - 1) // 2  # 24
H = 512
W = 512
C = 3
WC = W * C  # 1536
P = 128
HT = H // P  # 4 partition tiles per 512 dim
N_CORES = 8
IMGS = 8  # images per core
# float32r streams at 1/4 rate below 256-wide output; bf16 has no minimum
MIN_N = 256 if "float32r" in MM_DTYPE else 1

_CACHE: dict = {}


def _gauss_matrix() -> np.ndarray:
    """512x512 banded symmetric blur matrix A[i, j] = g[j - i + 24]."""
    r = np.arange(KSIZE, dtype=np.float32) - (KSIZE - 1) / 2.0
    g = np.exp(-(r * r) / (2.0 * SIGMA * SIGMA)).astype(np.float32)
    g = g / g.sum(dtype=np.float32)
    A = np.zeros((H, H), dtype=np.float32)
    for i in range(H):
        lo, hi = max(0, i - R), min(H, i + R + 1)
        A[i, lo:hi] = g[lo - i + R : hi - i + R]
    return A


def _bands():
    """Output-column window streamed for each 128-row block of A."""
    bands = [(0, H)]  # t=0: full width, start=True zero-fills the bank
    for t in range(1, HT):
        b0 = P * t - R
        b1 = min(H, P * t + P + R)  # true band, width <= 176
        if b1 - b0 < MIN_N:  # widen leftward (extra cols hit zeros of A)
            b0 = max(0, b1 - MIN_N)
        bands.append((b0, b1))
    return bands


def _build():
    # Bacc (not raw Bass): its compile() legalizes multi-sem waits down to
    # the 1-wait-per-instruction HW limit (generate_event_semaphores).
    nc = bacc.Bacc("TRN2", target_bir_lowering=False, debug=False,
                   num_devices=N_CORES)
    x_ext = nc.declare_dram_parameter("x", [IMGS, H, WC], mybir.dt.float32,
                                      isOutput=False)
    out_ext = nc.declare_dram_parameter("out", [IMGS, H, WC], mybir.dt.float32,
                                        isOutput=True)
    mm_dt = getattr(mybir.dt, MM_DTYPE)
    a_np = _gauss_matrix()
    if MM_DTYPE == "bfloat16":
        import ml_dtypes
        a_np = a_np.astype(ml_dtypes.bfloat16)
    a_dram = nc.inline_tensor(a_np, name="gmat")
    bands = _bands()
    # per-h-tile chunked I/O: [n, t, p, f] with contiguous 768 KB chunks
    x_ap = x_ext[:].rearrange("n (t p) f -> n t p f", p=P)
    out_ap = out_ext[:].rearrange("n (t p) f -> n t p f", p=P)

    with tile.TileContext(nc) as tc:
        from contextlib import ExitStack

        with ExitStack() as ctx:
            const_pool = ctx.enter_context(tc.tile_pool(name="const", bufs=1))
            x32_pool = ctx.enter_context(tc.tile_pool(name="x32p", bufs=8))
            x16_pool = ctx.enter_context(tc.tile_pool(name="x16p", bufs=2))
            y1_pool = ctx.enter_context(tc.tile_pool(name="y1p", bufs=2))
            z_pool = ctx.enter_context(tc.tile_pool(name="zp", bufs=6))
            ps1_pool = ctx.enter_context(
                tc.tile_pool(name="ps1p", bufs=4, space="PSUM"))
            ps2_pool = ctx.enter_context(
                tc.tile_pool(name="ps2p", bufs=4, space="PSUM"))

            # A row-blocks: g_sb[:, t, :] = A[128t : 128t+128, :]
            g_sb = const_pool.tile([P, HT, H], mm_dt)
            nc.sync.dma_start(
                out=g_sb[:], in_=a_dram[:].rearrange("(t p) h -> p t h", p=P))

            for n in range(IMGS):
                # in-DMAs chunked per h-tile on the sync queue; each chunk is
                # cast f32->bf16 (contiguous, DVE 2x mode) as it lands, so
                # pass-1 matmuls on block t can start before block t+1 loads.
                x16 = x16_pool.tile([P, HT, WC], mm_dt)
                for t in range(HT):
                    x32 = x32_pool.tile([P, WC], mybir.dt.float32)
                    nc.sync.dma_start(out=x32[:], in_=x_ap[n, t])
                    nc.vector.tensor_copy(x16[:, t, :], x32[:])
                # natural-layout view for strided (c,w) weight slices:
                # [p, t, w, c] -> lhsT free dim walks w (stride 6B) at fixed c
                x16v = x16[:].rearrange("p t (w c) -> p t w c", c=C)

                # ---- pass 1: blur along H; output Y1[(c,w), h] transposed
                y1 = y1_pool.tile([P, C, HT, H], mm_dt)
                for c in range(C):
                    for wt in range(HT):
                        ps1 = ps1_pool.tile([P, H], mybir.dt.float32)
                        for t in range(HT):
                            b0, b1 = bands[t]
                            nc.tensor.matmul(
                                ps1[:, b0:b1],
                                lhsT=x16v[:, t, wt * P:(wt + 1) * P, c],
                                rhs=g_sb[:, t, b0:b1],
                                start=(t == 0),
                                stop=(t == HT - 1),
                            )
                        nc.vector.tensor_copy(y1[:, c, wt, :], ps1[:])

                # ---- pass 2: blur along W; output Z[h, (w,c)] NHWC-ready
                for ht in range(HT):
                    z = z_pool.tile([P, WC], mybir.dt.float32)
                    for c in range(C):
                        ps2 = ps2_pool.tile([P, W], mybir.dt.float32)
                        for t in range(HT):
                            b0, b1 = bands[t]
                            nc.tensor.matmul(
                                ps2[:, b0:b1],
                                lhsT=y1[:, c, t, ht * P:(ht + 1) * P],
                                rhs=g_sb[:, t, b0:b1],
                                start=(t == 0),
                                stop=(t == HT - 1),
                            )
                        # eviction scatters channel c into interleaved (w,c)
                        zdst = z[:].rearrange("p (w c) -> p c w", c=C)[:, c, :]
                        if EVICT2_ENGINE == "scalar":
                            nc.scalar.activation(
                                zdst, ps2[:], mybir.ActivationFunctionType.Copy)
                        else:
                            nc.vector.tensor_copy(zdst, ps2[:])
                    # out-DMA per h-tile from the scalar queue: it directly
                    # follows this h-tile's evictions in ACT program order, so
                    # its wait is satisfied on issue and it never head-of-line
                    # blocks the sync queue's in-DMAs.
                    nc.scalar.dma_start(out=out_ap[n, ht], in_=z[:])

    nc.compile()
    return nc


def kernel(x: np.ndarray) -> np.ndarray:
    assert x.shape == (N_CORES * IMGS, H, W, C) and x.dtype == np.float32
    if "nc" not in _CACHE:
        _CACHE["nc"] = _build()
    nc = _CACHE["nc"]

    x = np.ascontiguousarray(x)
    in_maps = [
        {"x": x[i * IMGS:(i + 1) * IMGS].reshape(IMGS, H, WC)}
        for i in range(N_CORES)
    ]
    trace = os.environ.get("BLUR_TRACE", "0") == "1"
    res = run_bass_kernel_spmd(nc, in_maps, core_ids=list(range(N_CORES)),
                               trace=trace)
    _CACHE["last_results"] = res
    out = np.concatenate([res.results[i]["out"] for i in range(N_CORES)], axis=0)
    return np.ascontiguousarray(out.reshape(N_CORES * IMGS, H, W, C))


if __name__ == "__main__":
    xs = np.random.randn(64, H, W, C).astype(np.float32)
    y = kernel(xs)
    print(y.shape, y.dtype)
